# revision 1
# baseline (speedup 1.0000x reference)
"""Trainium2 Bass kernel for a single-layer attention module (RMSNorm + QKV +
RoPE + causal attention over a KV cache + output projection), tensor-parallel
over 8 NeuronCores (4 heads each), per-head AllGather of attention outputs,
and per-core output-column blocks of the final projection.

Self-contained: takes FULL inputs, returns the FULL [1024, 4096] f32 output.
"""

import sys

sys.path.insert(0, "/opt/trn_rl_repo")

import numpy as np
import ml_dtypes

import concourse.bass as bass  # noqa: F401
import concourse.bacc as bacc
import concourse.tile as tile
from concourse import mybir
from concourse import bass_utils

BF16 = ml_dtypes.bfloat16
F32 = np.float32

N_CORES = 8
D, H, HD, S, C = 4096, 32, 128, 1024, 2048
T = C + S          # 3072 total keys
HL = H // N_CORES  # 4 heads per core
OC = HL * HD       # 512 local attention features per core
NDK = D // 128     # 32 contraction tiles over D
NTC = C // 128     # 16 cache t-tiles
NTN = S // 128     # 8 new-key t-tiles
EPS = 1e-6
THETA = 10000.0

bf = mybir.dt.bfloat16
f32 = mybir.dt.float32


def _build_nc():
    nc = bacc.Bacc("TRN2", target_bir_lowering=False, debug=False,
                   num_devices=N_CORES)

    # ---- DRAM I/O ----
    xs_t = nc.dram_tensor("xs_t", [128, NDK * S], bf, kind="ExternalInput")
    wq_col = nc.dram_tensor("wq_col", [HL, 128, NDK * 128], bf, kind="ExternalInput")
    wk_col = nc.dram_tensor("wk_col", [HL, 128, NDK * 128], bf, kind="ExternalInput")
    wv_col = nc.dram_tensor("wv_col", [HL, 128, NDK * 128], bf, kind="ExternalInput")
    wo_blk = nc.dram_tensor("wo_blk", [HL, 128, 8 * OC], bf, kind="ExternalInput")
    ckt = nc.dram_tensor("ckt", [HL, 128, C], bf, kind="ExternalInput")
    cvr = nc.dram_tensor("cvr", [HL, 128, C], bf, kind="ExternalInput")
    cosT = nc.dram_tensor("cosT", [128, S], bf, kind="ExternalInput")
    sinT = nc.dram_tensor("sinT", [128, S], bf, kind="ExternalInput")
    maskW = nc.dram_tensor("maskW", [128, 2 * S], bf, kind="ExternalInput")
    ones_d = nc.dram_tensor("ones_d", [128, 128], bf, kind="ExternalInput")
    id_d = nc.dram_tensor("id_d", [128, 128], bf, kind="ExternalInput")
    # y stored transposed ([outcol, s]); host transposes back
    y = nc.dram_tensor("y", [OC, S], f32, kind="ExternalOutput")

    with tile.TileContext(nc) as tc:
        with (
            tc.tile_pool(name="const", bufs=1) as cpool,
            tc.tile_pool(name="qk", bufs=1) as qkpool,
            tc.tile_pool(name="kv", bufs=4) as kvpool,
            tc.tile_pool(name="exp", bufs=6) as epool,
            tc.tile_pool(name="att", bufs=1) as apool,
            tc.tile_pool(name="rec", bufs=4) as recpool,
            tc.tile_pool(name="dram", bufs=1, space="DRAM") as dpool,
        ):
            # persistent per-head results
            qr = qkpool.tile([128, HL * S], bf, name="qr")
            kr = qkpool.tile([128, HL * S], bf, name="kr")
            v_sb = qkpool.tile([128, HL * S], bf, name="v_sb")
            attnT = apool.tile([128, HL * S], bf, name="attnT")
            ag_in = [dpool.tile([128, S], bf, name=f"ag_in{h}") for h in range(HL)]
            ag_out = [dpool.tile([N_CORES * 128, S], bf, name=f"ag_out{h}",
                                 addr_space="Shared") for h in range(HL)]

            # =========== scope A: load xs, RMSNorm stats + projections ===========
            # Projections run on RAW xs^T (the 1/rms scale is applied after the
            # matmul, by linearity), so PE work starts as soon as DMA lands.
            with (
                tc.tile_pool(name="xs", bufs=16) as xpool,
                tc.tile_pool(name="sq", bufs=4) as sqpool,
                tc.tile_pool(name="nrm", bufs=1) as npool,
                tc.tile_pool(name="wcol", bufs=2) as wpool,
                tc.tile_pool(name="hh", bufs=2) as hpool,
                tc.tile_pool(name="rope", bufs=2) as rpool,
                tc.tile_pool(name="psA", bufs=2, space="PSUM") as psA,
                tc.tile_pool(name="psT", bufs=2, space="PSUM") as psT,
            ):
                # tiny constants first, then xs (16 small chunks so PE work
                # starts early), then the big tables
                ones_t = cpool.tile([128, 128], bf, name="ones_t")
                nc.sync.dma_start(ones_t[:], ones_d[:])
                id_t = cpool.tile([128, 128], bf, name="id_t")
                nc.sync.dma_start(id_t[:], id_d[:])
                xs_ch = []
                for g in range(16):
                    xc = xpool.tile([128, 2 * S], bf, name="xs_ch")
                    nc.sync.dma_start(xc[:], xs_t[:, g * 2 * S:(g + 1) * 2 * S])
                    xs_ch.append(xc)

                def xs_v(dk):  # [128, S] view of raw xs^T d-tile dk
                    return xs_ch[dk // 2][:, (dk % 2) * S:(dk % 2 + 1) * S]

                cos_t = cpool.tile([128, S], bf, name="cos_t")
                nc.sync.dma_start(cos_t[:], cosT[:])
                sin_t = cpool.tile([128, S], bf, name="sin_t")
                nc.sync.dma_start(sin_t[:], sinT[:])
                mask_t = cpool.tile([128, 2 * S], bf, name="mask_t")
                nc.sync.dma_start(mask_t[:], maskW[:])

                ps_ss = psA.tile([128, S], f32, name="psp")
                rsq_box = {}

                def emit_norm_dk(dk):
                    sqt = sqpool.tile([128, S], bf, name="sqt")
                    nc.vector.tensor_mul(sqt[:], xs_v(dk), xs_v(dk))
                    for sc in range(2):
                        nc.tensor.matmul(
                            ps_ss[:, sc * 512:(sc + 1) * 512],
                            ones_t[:], sqt[:, sc * 512:(sc + 1) * 512],
                            start=(dk == 0), stop=(dk == NDK - 1))

                def emit_rsq():
                    ssum = npool.tile([128, S], f32, name="ssum")
                    nc.scalar.activation(ssum[:], ps_ss[:],
                                         mybir.ActivationFunctionType.Copy,
                                         bias=EPS, scale=1.0 / D)
                    rcp = npool.tile([128, S], f32, name="rcp")
                    nc.vector.reciprocal_approx_fast(rcp[:], ssum[:])
                    rsq = npool.tile([128, S], bf, name="rsq")
                    nc.scalar.sqrt(rsq[:], rcp[:])
                    rsq_box["rsq"] = rsq

                def rope(dst, src):
                    # dst = src * cos2 + rot(src) * sin2,
                    # rot(src) = [src_hi; src_lo] via SBUF->SBUF DMA
                    rot = rpool.tile([128, S], bf, name="rot")
                    nc.sync.dma_start(rot[0:64, :], src[64:128, :])
                    nc.sync.dma_start(rot[64:128, :], src[0:64, :])
                    ta = rpool.tile([128, S], bf, name="ta")
                    nc.vector.tensor_mul(ta[:], src[:], cos_t[:])
                    tb = rpool.tile([128, S], bf, name="tb")
                    nc.vector.tensor_mul(tb[:], rot[:], sin_t[:])
                    nc.vector.tensor_add(dst[:], ta[:], tb[:])

                first = True
                for which, wsrc in (("q", wq_col), ("k", wk_col), ("v", wv_col)):
                    for h in range(HL):
                        wcol = wpool.tile([128, NDK * 128], bf, name="wcol")
                        nc.sync.dma_start(wcol[:], wsrc[h])
                        psp = psA.tile([128, S], f32, name="psp")
                        for dk in range(NDK):
                            if first:
                                emit_norm_dk(dk)
                            for sc in range(2):
                                nc.tensor.matmul(
                                    psp[:, sc * 512:(sc + 1) * 512],
                                    wcol[:, dk * 128:(dk + 1) * 128],
                                    xs_v(dk)[:, sc * 512:(sc + 1) * 512],
                                    start=(dk == 0), stop=(dk == NDK - 1))
                        if first:
                            emit_rsq()
                            first = False
                        hh = hpool.tile([128, S], bf, name="hh")
                        nc.scalar.copy(hh[:], psp[:])
                        hh2 = hpool.tile([128, S], bf, name="hh2")
                        nc.vector.tensor_mul(hh2[:], hh[:], rsq_box["rsq"][:])
                        if which == "q":
                            rope(qr[:, h * S:(h + 1) * S], hh2[:])
                        elif which == "k":
                            rope(kr[:, h * S:(h + 1) * S], hh2[:])
                        else:
                            for tj in range(NTN):
                                ptr = psT.tile([128, 128], bf, name="ptr")
                                nc.tensor.transpose(
                                    ptr[:], hh2[:, tj * 128:(tj + 1) * 128], id_t[:])
                                nc.scalar.copy(
                                    v_sb[:, h * S + tj * 128: h * S + (tj + 1) * 128],
                                    ptr[:])

            # =========== scope B: attention (software-pipelined) ===========
            # New-key tiles first (their mask-multiply hides mid-head), cache
            # tiles last so each head drains through a short S->exp->PV chain.
            with (
                tc.tile_pool(name="psS", bufs=2, space="PSUM") as psS,
                tc.tile_pool(name="psDen", bufs=2, space="PSUM") as psDen,
                tc.tile_pool(name="psO", bufs=4, space="PSUM") as psO,
            ):
                for h in range(HL):
                    ck_sb = kvpool.tile([128, C], bf, name="ck_sb")
                    nc.sync.dma_start(ck_sb[:], ckt[h])
                    cv_sb = kvpool.tile([128, C], bf, name="cv_sb")
                    nc.sync.dma_start(cv_sb[:], cvr[h])
                    qh = qr[:, h * S:(h + 1) * S]
                    den = [psDen.tile([128, 512], f32, name="den") for _ in range(2)]
                    Oc = [psO.tile([128, 512], f32, name="Oc") for _ in range(2)]

                    def lts_ltv(ti):
                        if ti < NTC:
                            return (ck_sb[:, ti * 128:(ti + 1) * 128],
                                    cv_sb[:, ti * 128:(ti + 1) * 128])
                        tn = ti - NTC
                        return (kr[:, h * S + tn * 128: h * S + (tn + 1) * 128],
                                v_sb[:, h * S + tn * 128: h * S + (tn + 1) * 128])

                    groups = []
                    for ti in range(NTC + NTN):
                        for sc in range(2):
                            if ti >= NTC and (ti - NTC) * 128 > sc * 512 + 511:
                                continue       # fully masked tile
                            groups.append((ti, sc))
                    gfirst = {}
                    for g in groups:
                        gfirst.setdefault(g[1], g)
                    glast = {sc: [g for g in groups if g[1] == sc][-1]
                             for sc in range(2)}

                    def emit_epilogue(sc):
                        rec = recpool.tile([128, 512], f32, name="rec")
                        nc.vector.reciprocal_approx_fast(rec[:], den[sc][:])
                        nc.vector.tensor_mul(
                            attnT[:, h * S + sc * 512: h * S + (sc + 1) * 512],
                            Oc[sc][:], rec[:])
                        nc.sync.dma_start(
                            ag_in[h][:, sc * 512:(sc + 1) * 512],
                            attnT[:, h * S + sc * 512: h * S + (sc + 1) * 512])

                    def emit_den_O(g, e_t):
                        ti, sc = g
                        _, lv = lts_ltv(ti)
                        nc.tensor.matmul(den[sc][:], ones_t[:], e_t[:],
                                         start=(g == gfirst[sc]),
                                         stop=(g == glast[sc]))
                        nc.tensor.matmul(Oc[sc][:], lv, e_t[:],
                                         start=(g == gfirst[sc]),
                                         stop=(g == glast[sc]))
                        if g == glast[sc]:
                            emit_epilogue(sc)

                    from collections import deque
                    pend = deque()  # (group, e_tile), two steps behind
                    for g in groups:
                        ti, sc = g
                        ls, _ = lts_ltv(ti)
                        ps = psS.tile([128, 512], f32, name="ps")
                        nc.tensor.matmul(
                            ps[:], ls, qh[:, sc * 512:(sc + 1) * 512],
                            start=True, stop=True)
                        e = epool.tile([128, 512], bf, name="e")
                        nc.scalar.activation(
                            e[:], ps[:], mybir.ActivationFunctionType.Exp)
                        if ti >= NTC and (ti - NTC) * 128 + 127 > sc * 512:
                            off = S - (ti - NTC) * 128 + sc * 512
                            nc.vector.tensor_mul(
                                e[:], e[:], mask_t[:, off:off + 512])
                        if len(pend) >= 2:
                            emit_den_O(*pend.popleft())
                        pend.append((g, e))
                    while pend:
                        emit_den_O(*pend.popleft())

                    # per-head AllGather right after this head's output DMA
                    nc.gpsimd.collective_compute(
                        "AllGather", mybir.AluOpType.bypass,
                        replica_groups=[list(range(N_CORES))],
                        ins=[ag_in[h][:]], outs=[ag_out[h][:]])

            # =========== scope C: output projection (y^T orientation) ===========
            # o-tile (= global head g = 4r + h_local) rows live in ag_out[h][r].
            # lhsT = wo column tiles, rhs = gathered attn^T rows, N=1024.
            with (
                tc.tile_pool(name="psY", bufs=4, space="PSUM") as psY,
                tc.tile_pool(name="ag", bufs=4) as agpool,
                tc.tile_pool(name="wo", bufs=4) as wopool,
                tc.tile_pool(name="yout", bufs=2) as ypool,
            ):
                ps_y = [psY.tile([128, S], f32, name="ps_y") for _ in range(4)]
                ag_sbs, wo_sbs = [], []
                for h in range(HL):
                    ag_sb = agpool.tile([128, 8 * S], bf, name="ag_sb")
                    nc.sync.dma_start(
                        ag_sb[:].rearrange("p (r s) -> p r s", r=8),
                        ag_out[h][:].rearrange("(r p) s -> p r s", p=128))
                    ag_sbs.append(ag_sb)
                    wo_sb = wopool.tile([128, 8 * OC], bf, name="wo_sb")
                    nc.sync.dma_start(wo_sb[:], wo_blk[h])
                    wo_sbs.append(wo_sb)
                n_ot = 0
                for h in range(HL):
                    ag_sb = ag_sbs[h]
                    wo_sb = wo_sbs[h]
                    for r in range(8):
                        for oc8 in range(4):
                            for sc in range(2):
                                nc.tensor.matmul(
                                    ps_y[oc8][:, sc * 512:(sc + 1) * 512],
                                    wo_sb[:, (r * 4 + oc8) * 128:
                                          (r * 4 + oc8 + 1) * 128],
                                    ag_sb[:, r * S + sc * 512:
                                          r * S + (sc + 1) * 512],
                                    start=(n_ot == 0), stop=(n_ot == 31))
                        n_ot += 1
                for oc8 in range(4):
                    ysb = ypool.tile([128, S], f32, name="ysb")
                    nc.scalar.copy(ysb[:], ps_y[oc8][:])
                    nc.sync.dma_start(y[oc8 * 128:(oc8 + 1) * 128, :], ysb[:])

    nc.compile()
    return nc


def _host_prep(xs, cache_k, cache_v, norm_w, wq, wk, wv, wo):
    """Build the 8 per-core input maps (all layout work done on host)."""
    xs = np.asarray(xs, F32)
    cache_k = np.asarray(cache_k, F32)
    cache_v = np.asarray(cache_v, F32)
    norm_w = np.asarray(norm_w, F32)
    wq, wk, wv, wo = (np.asarray(w, F32) for w in (wq, wk, wv, wo))

    # xs^T tiled: [128, dk*S]
    xs_t = np.ascontiguousarray(
        xs.T.reshape(NDK, 128, S).transpose(1, 0, 2).reshape(128, NDK * S)
    ).astype(BF16)

    # RoPE tables (positions C..C+S-1), transposed [freq, s]
    half = HD // 2
    inv_freq = 1.0 / (THETA ** (np.arange(0, half, dtype=np.float64) * 2.0 / HD))
    pos = np.arange(S, dtype=np.float64) + C
    ang = np.outer(pos, inv_freq)          # [S, 64]
    cos1 = np.cos(ang).T.astype(F32)       # [64, S]
    sin1 = np.sin(ang).T.astype(F32)
    cosT = np.vstack([cos1, cos1]).astype(BF16)          # [128, S]
    sinT = np.vstack([-sin1, sin1]).astype(BF16)         # rotate-half signs

    # sliding causal mask window: W[p, j] = 1 iff j >= S + p
    jj = np.arange(2 * S)[None, :]
    pp = np.arange(128)[:, None]
    maskW = (jj >= S + pp).astype(F32).astype(BF16)

    ones_d = np.ones((128, 128), F32).astype(BF16)
    id_d = np.eye(128, dtype=F32).astype(BF16)

    sc_q = F32(1.0) / np.sqrt(F32(HD))

    in_maps = []
    for c in range(N_CORES):
        osl = slice(OC * c, OC * (c + 1))
        hsl = slice(HL * c, HL * (c + 1))
        # fold norm_w into wq/wk/wv; fold 1/sqrt(HD) into wq
        wq_c = (wq[osl] * norm_w[None, :]) * sc_q   # [512, 4096]
        wk_c = wk[osl] * norm_w[None, :]
        wv_c = wv[osl] * norm_w[None, :]

        def col_layout(w_c):
            # [HL, 128, NDK*128]: [h, p, dk*128 + j] = w_c[h*128+j, dk*128+p]
            m = w_c.reshape(HL, 128, NDK, 128)          # [h, j, dk, p]
            return np.ascontiguousarray(
                m.transpose(0, 3, 2, 1).reshape(HL, 128, NDK * 128)).astype(BF16)

        wq_col = col_layout(wq_c)
        wk_col = col_layout(wk_c)
        wv_col = col_layout(wv_c)

        # wo block for y^T: [h, p, (r*4 + oc8)*128 + j] =
        #   wo[OC*c + oc8*128 + j, (4r + h)*128 + p]
        wo_c = wo[osl]                                  # [512, 4096]
        m = wo_c.reshape(4, 128, 8, HL, 128)            # [oc8, j, r, h, p]
        wo_blk = np.ascontiguousarray(
            m.transpose(3, 4, 2, 0, 1).reshape(HL, 128, 8 * OC)).astype(BF16)

        # cache K^T per head: [h, p(hd), t]
        ck = np.ascontiguousarray(
            cache_k[:, hsl, :].transpose(1, 2, 0)).astype(BF16)   # [HL, 128, C]
        # cache V tiles: [h, p(t%128), ti*128 + hd]
        cv = np.ascontiguousarray(
            cache_v[:, hsl, :].reshape(NTC, 128, HL, HD)
            .transpose(2, 1, 0, 3).reshape(HL, 128, C)).astype(BF16)

        in_maps.append({
            "xs_t": xs_t, "wq_col": wq_col, "wk_col": wk_col, "wv_col": wv_col,
            "wo_blk": wo_blk, "ckt": ck, "cvr": cv,
            "cosT": cosT, "sinT": sinT, "maskW": maskW,
            "ones_d": ones_d, "id_d": id_d,
        })
    return in_maps


_NC_CACHE = {}


def kernel(xs, cache_k, cache_v, norm_w, wq, wk, wv, wo, _trace=False):
    if "nc" not in _NC_CACHE:
        _NC_CACHE["nc"] = _build_nc()
    nc = _NC_CACHE["nc"]
    in_maps = _host_prep(xs, cache_k, cache_v, norm_w, wq, wk, wv, wo)
    res = bass_utils.run_bass_kernel_spmd(
        nc, in_maps, core_ids=list(range(N_CORES)), trace=_trace)
    out = np.concatenate(
        [res.results[c]["y"].T for c in range(N_CORES)], axis=1)
    out = np.ascontiguousarray(out)
    if _trace:
        kernel.last_exec_time_ns = res.exec_time_ns
        kernel.last_results = res
    return out



# revision 4
# speedup vs baseline: 1.1138x; 1.1138x over previous
"""Trainium2 Bass kernel for a single-layer attention module (RMSNorm + QKV +
RoPE + causal attention over a KV cache + output projection), tensor-parallel
over 8 NeuronCores (4 heads each), per-head AllGather of attention outputs,
and per-core output-column blocks of the final projection.

Merged-pipeline version: QKV projections of head h+1 and output-projection
matmuls are interleaved into attention head h's PE stream (hiding exp latency
and the collectives), softmax denominators are accumulated on the vector
engine (pair chains) instead of ones-matmuls, RMSNorm statistics are
accumulated on the vector engine, and score/PV matmuls use exact causal
widths.

Self-contained: takes FULL inputs, returns the FULL [1024, 4096] f32 output.
"""

import sys

sys.path.insert(0, "/opt/trn_rl_repo")

from collections import deque
from itertools import islice

import numpy as np
import ml_dtypes

import concourse.bass as bass  # noqa: F401
import concourse.bacc as bacc
import concourse.tile as tile
from concourse import mybir
from concourse import bass_utils

BF16 = ml_dtypes.bfloat16
F32 = np.float32

N_CORES = 8
D, H, HD, S, C = 4096, 32, 128, 1024, 2048
T = C + S          # 3072 total keys
HL = H // N_CORES  # 4 heads per core
OC = HL * HD       # 512 local attention features per core
NDK = D // 128     # 32 contraction tiles over D
NTC = C // 128     # 16 cache t-tiles
NTN = S // 128     # 8 new-key t-tiles
EPS = 1e-6
THETA = 10000.0

bf = mybir.dt.bfloat16
f32 = mybir.dt.float32


def _build_nc():
    nc = bacc.Bacc("TRN2", target_bir_lowering=False, debug=False,
                   num_devices=N_CORES)

    # ---- DRAM I/O ----
    xs_t = nc.dram_tensor("xs_t", [128, NDK * S], bf, kind="ExternalInput")
    wq_col = nc.dram_tensor("wq_col", [HL, 128, NDK * 128], bf, kind="ExternalInput")
    wk_col = nc.dram_tensor("wk_col", [HL, 128, NDK * 128], bf, kind="ExternalInput")
    wv_col = nc.dram_tensor("wv_col", [HL, 128, NDK * 128], bf, kind="ExternalInput")
    wo_blk = nc.dram_tensor("wo_blk", [HL, 128, 8 * OC], bf, kind="ExternalInput")
    ckt = nc.dram_tensor("ckt", [HL, 128, C], bf, kind="ExternalInput")
    cvr = nc.dram_tensor("cvr", [HL, 128, C], bf, kind="ExternalInput")
    cosT = nc.dram_tensor("cosT", [128, S], bf, kind="ExternalInput")
    sinT = nc.dram_tensor("sinT", [128, S], bf, kind="ExternalInput")
    triW = nc.dram_tensor("triW", [128, 128], bf, kind="ExternalInput")
    ones_d = nc.dram_tensor("ones_d", [128, 128], bf, kind="ExternalInput")
    id_d = nc.dram_tensor("id_d", [128, 128], bf, kind="ExternalInput")
    # y stored transposed ([outcol, s]); host transposes back
    y = nc.dram_tensor("y", [OC, S], f32, kind="ExternalOutput")

    with tile.TileContext(nc) as tc:
        with (
            tc.tile_pool(name="const", bufs=1) as cpool,
            tc.tile_pool(name="qk", bufs=2) as qkpool,
            tc.tile_pool(name="att", bufs=2) as apool,
            tc.tile_pool(name="kv", bufs=2) as kvpool,
            tc.tile_pool(name="exp", bufs=6) as epool,
            tc.tile_pool(name="accp", bufs=6) as accpool,
            tc.tile_pool(name="rec", bufs=2) as recpool,
            tc.tile_pool(name="wo", bufs=4) as wopool,
            tc.tile_pool(name="dram", bufs=1, space="DRAM") as dpool,
            tc.tile_pool(name="psp", bufs=2, space="PSUM") as psp_pool,
        ):
            # ---- constants ----
            ones_t = cpool.tile([128, 128], bf, name="ones_t")
            nc.sync.dma_start(ones_t[:], ones_d[:])
            id_t = cpool.tile([128, 128], bf, name="id_t")
            nc.sync.dma_start(id_t[:], id_d[:])
            tri_t = cpool.tile([128, 128], bf, name="tri_t")
            nc.sync.dma_start(tri_t[:], triW[:])

            # per-head tile handles (ring depth 2: head h live + h+1 building)
            qr = [None] * HL
            kr = [None] * HL
            vsb = [None] * HL
            cks = [None] * HL
            cvs = [None] * HL
            wos = [None] * HL
            # collective dram buffers
            ag_in = [dpool.tile([128, S], bf, name=f"ag_in{h}") for h in range(3)]
            ag_out = [dpool.tile([N_CORES * 128, S], bf, name=f"ag_out{h}",
                                 addr_space="Shared") for h in range(3)]
            ag_in3 = [dpool.tile([128, 512], bf, name=f"ag_in3_{s}")
                      for s in range(2)]
            ag_out3 = [dpool.tile([N_CORES * 128, 512], bf, name=f"ag_out3_{s}",
                                  addr_space="Shared") for s in range(2)]

            box = {}       # rsq tile, transpose pool, transpose tile-name

            def groups_for(sc):
                # (kind, idx, off0): cache tiles full width, then new-key tiles
                # at exact causal width
                gs = [("c", ti, 0) for ti in range(NTC)]
                for tn in range(NTN):
                    if tn * 128 < (sc + 1) * 512:
                        gs.append(("n", tn, max(0, tn * 128 - sc * 512)))
                return gs

            def att_head(h, filler, psS, psO, last):
                """Emit attention for head h, draining filler between groups."""
                if h + 1 < HL:
                    cks[h + 1] = kvpool.tile([128, C], bf, name="ck_sb")
                    nc.sync.dma_start(cks[h + 1][:], ckt[h + 1])
                    cvs[h + 1] = kvpool.tile([128, C], bf, name="cv_sb")
                    nc.sync.dma_start(cvs[h + 1][:], cvr[h + 1])
                attnT = apool.tile([128, S], bf, name="attnT")

                def drain(k):
                    for _ in range(k):
                        if next(filler, None) is None:
                            break

                for sc in range(2):
                    gs = groups_for(sc)
                    oc_t = psO.tile([128, 512], f32, name="oc")
                    partials = [None, None]
                    stash = [None, None]
                    nnew = [0]

                    def retire(g, e):
                        kind, idx, off0 = g
                        lv = (cvs[h][:, idx * 128:(idx + 1) * 128]
                              if kind == "c" else
                              vsb[h][:, idx * 128:(idx + 1) * 128])
                        nc.tensor.matmul(
                            oc_t[:, off0:512], lv, e[:, off0:512],
                            start=(g == gs[0]), stop=(g == gs[-1]))
                        # softmax denominator accumulation on DVE
                        if kind == "c":
                            j = idx % 2
                            if partials[j] is None:
                                if stash[j] is None:
                                    stash[j] = e
                                else:
                                    p = accpool.tile([128, 512], bf, name="accp")
                                    nc.vector.tensor_add(p[:], stash[j][:], e[:])
                                    partials[j] = p
                                    stash[j] = None
                            else:
                                nc.vector.tensor_add(
                                    partials[j][:], partials[j][:], e[:])
                        else:
                            j = nnew[0] % 2
                            nnew[0] += 1
                            nc.vector.tensor_add(
                                partials[j][:, off0:512],
                                partials[j][:, off0:512], e[:, off0:512])

                    pend = deque()
                    for g in gs:
                        kind, idx, off0 = g
                        lk = (cks[h][:, idx * 128:(idx + 1) * 128]
                              if kind == "c" else
                              kr[h][:, idx * 128:(idx + 1) * 128])
                        ps = psS.tile([128, 512], f32, name="ps")
                        nc.tensor.matmul(
                            ps[:, off0:512], lk,
                            qr[h][:, sc * 512 + off0:(sc + 1) * 512],
                            start=True, stop=True)
                        e = epool.tile([128, 512], bf, name="e")
                        nc.scalar.activation(
                            e[:, off0:512], ps[:, off0:512],
                            mybir.ActivationFunctionType.Exp)
                        if kind == "n" and idx * 128 >= sc * 512:
                            nc.vector.tensor_mul(
                                e[:, off0:off0 + 128], e[:, off0:off0 + 128],
                                tri_t[:])
                        drain(5 if not last else 3)
                        if len(pend) >= 2:
                            retire(*pend.popleft())
                        pend.append((g, e))
                    while pend:
                        retire(*pend.popleft())

                    # den = colsum(partial0) + colsum(partial1) via 2 ones-MMs
                    den_ps = psS.tile([128, 512], f32, name="ps")
                    nc.tensor.matmul(den_ps[:], ones_t[:], partials[0][:],
                                     start=True, stop=False)
                    nc.tensor.matmul(den_ps[:], ones_t[:], partials[1][:],
                                     start=False, stop=True)
                    rec = recpool.tile([128, 512], f32, name="rec")
                    nc.vector.reciprocal_approx_fast(rec[:], den_ps[:])
                    nc.vector.tensor_mul(
                        attnT[:, sc * 512:(sc + 1) * 512], oc_t[:], rec[:])
                    if not last:
                        nc.sync.dma_start(
                            ag_in[h][:, sc * 512:(sc + 1) * 512],
                            attnT[:, sc * 512:(sc + 1) * 512])
                    else:
                        nc.sync.dma_start(
                            ag_in3[sc][:], attnT[:, sc * 512:(sc + 1) * 512])
                        nc.gpsimd.collective_compute(
                            "AllGather", mybir.AluOpType.bypass,
                            replica_groups=[list(range(N_CORES))],
                            ins=[ag_in3[sc][:]], outs=[ag_out3[sc][:]])
                if not last:
                    nc.gpsimd.collective_compute(
                        "AllGather", mybir.AluOpType.bypass,
                        replica_groups=[list(range(N_CORES))],
                        ins=[ag_in[h][:]], outs=[ag_out[h][:]])

            # =============== pre-region: xs + QKV head 0 (+ norm) ===============
            with (
                tc.tile_pool(name="xs", bufs=16) as xpool,
                tc.tile_pool(name="wcol", bufs=2) as wpool,
                tc.tile_pool(name="hh", bufs=2) as hpool,
                tc.tile_pool(name="rope", bufs=1) as rpool,
            ):
                # first weight column so PE work starts ASAP, then xs
                wcol0 = wpool.tile([128, NDK * 128], bf, name="wcol")
                nc.sync.dma_start(wcol0[:], wq_col[0])
                xs_ch = []
                for g in range(16):
                    xc = xpool.tile([128, 2 * S], bf, name="xs_ch")
                    nc.sync.dma_start(xc[:], xs_t[:, g * 2 * S:(g + 1) * 2 * S])
                    xs_ch.append(xc)

                def xs_v(dk):  # [128, S] view of raw xs^T d-tile dk
                    return xs_ch[dk // 2][:, (dk % 2) * S:(dk % 2 + 1) * S]

                cos_t = cpool.tile([128, S], bf, name="cos_t")
                nc.sync.dma_start(cos_t[:], cosT[:])
                sin_t = cpool.tile([128, S], bf, name="sin_t")
                nc.sync.dma_start(sin_t[:], sinT[:])
                # head-0 caches early (needed right at ATT_0)
                cks[0] = kvpool.tile([128, C], bf, name="ck_sb")
                nc.sync.dma_start(cks[0][:], ckt[0])
                cvs[0] = kvpool.tile([128, C], bf, name="cv_sb")
                nc.sync.dma_start(cvs[0][:], cvr[0])

                def rope(dst, src):
                    # dst = src * cos2 + rot(src) * sin2
                    rot = rpool.tile([128, S], bf, name="rot")
                    nc.sync.dma_start(rot[0:64, :], src[64:128, :])
                    nc.sync.dma_start(rot[64:128, :], src[0:64, :])
                    ta = rpool.tile([128, S], bf, name="ta")
                    nc.vector.tensor_mul(ta[:], src[:], cos_t[:])
                    tb = rpool.tile([128, S], bf, name="tb")
                    nc.vector.tensor_mul(tb[:], rot[:], sin_t[:])
                    nc.vector.tensor_add(dst[:], ta[:], tb[:])

                def emit_qkv_head(h):
                    """Generator: emits QKV for head h, yields after each PE op."""
                    qr[h] = qkpool.tile([128, S], bf, name="qr")
                    kr[h] = qkpool.tile([128, S], bf, name="kr")
                    vsb[h] = qkpool.tile([128, S], bf, name="vsb")
                    for which, wsrc in (("q", wq_col), ("k", wk_col), ("v", wv_col)):
                        if which == "q" and h == 0:
                            wc = wcol0
                        else:
                            wc = wpool.tile([128, NDK * 128], bf, name="wcol")
                            nc.sync.dma_start(wc[:], wsrc[h])
                        psp = psp_pool.tile([128, S], f32, name="psp")
                        for dk in range(NDK):
                            if h == 0 and which == "q":
                                box["norm_dk"](dk)
                            for scc in range(2):
                                nc.tensor.matmul(
                                    psp[:, scc * 512:(scc + 1) * 512],
                                    wc[:, dk * 128:(dk + 1) * 128],
                                    xs_v(dk)[:, scc * 512:(scc + 1) * 512],
                                    start=(dk == 0), stop=(dk == NDK - 1))
                                yield
                        if h == 0 and which == "q":
                            box["rsq_emit"]()
                        hh2 = hpool.tile([128, S], bf, name="hh2")
                        nc.vector.tensor_mul(hh2[:], psp[:], box["rsq"][:])
                        if which == "q":
                            rope(qr[h][:], hh2)
                        elif which == "k":
                            rope(kr[h][:], hh2)
                        else:
                            psT, ptr_name = box["psT"]
                            for tj in range(NTN):
                                ptr = psT.tile([128, 128], bf, name=ptr_name)
                                nc.tensor.transpose(
                                    ptr[:], hh2[:, tj * 128:(tj + 1) * 128],
                                    id_t[:])
                                nc.scalar.copy(
                                    vsb[h][:, tj * 128:(tj + 1) * 128], ptr[:])
                                yield

                # ---- RMSNorm stats: DVE squares + 2 f32 chains, 2 PE MMs ----
                gen0 = emit_qkv_head(0)
                with (
                    tc.tile_pool(name="nrm", bufs=1) as npool,
                    tc.tile_pool(name="sq", bufs=4) as sqpool,
                    tc.tile_pool(name="psn", bufs=1, space="PSUM") as psn,
                ):
                    naccs = [None, None]
                    stash_sq = [None, None]

                    def norm_dk(dk):
                        sqt = sqpool.tile([128, S], bf, name="sqt")
                        nc.vector.tensor_mul(sqt[:], xs_v(dk), xs_v(dk))
                        c = dk % 2
                        if naccs[c] is None:
                            if stash_sq[c] is None:
                                stash_sq[c] = sqt
                            else:
                                acc = npool.tile([128, S], f32, name=f"nacc{c}")
                                nc.vector.tensor_add(
                                    acc[:], stash_sq[c][:], sqt[:])
                                naccs[c] = acc
                                stash_sq[c] = None
                        else:
                            nc.vector.tensor_add(naccs[c][:], naccs[c][:], sqt[:])

                    def rsq_emit():
                        nacc_bf = npool.tile([128, S], bf, name="nacc_bf")
                        nc.vector.tensor_add(nacc_bf[:], naccs[0][:], naccs[1][:])
                        ps_ss = psn.tile([128, S], f32, name="ps_ss")
                        for scc in range(2):
                            nc.tensor.matmul(
                                ps_ss[:, scc * 512:(scc + 1) * 512],
                                ones_t[:], nacc_bf[:, scc * 512:(scc + 1) * 512],
                                start=True, stop=True)
                        ssum = npool.tile([128, S], f32, name="ssum")
                        nc.scalar.activation(ssum[:], ps_ss[:],
                                             mybir.ActivationFunctionType.Copy,
                                             bias=EPS, scale=1.0 / D)
                        rcp = npool.tile([128, S], f32, name="rcp")
                        nc.vector.reciprocal_approx_fast(rcp[:], ssum[:])
                        rsq = cpool.tile([128, S], bf, name="rsq")
                        nc.scalar.sqrt(rsq[:], rcp[:])
                        box["rsq"] = rsq

                    box["norm_dk"] = norm_dk
                    box["rsq_emit"] = rsq_emit
                    # stage 1: q-head-0 projection + norm stats (65 resumes
                    # cover the 64 q matmuls plus the rsq/epilogue emission)
                    for _ in islice(gen0, 65):
                        pass

                # stage 2: k0/v0 with a transient transpose PSUM pool
                with tc.tile_pool(name="psT0", bufs=2, space="PSUM") as psT0:
                    box["psT"] = (psT0, "ptr")
                    for _ in gen0:
                        pass

                # =============== heads 0-2: attention ⊗ QKV(h+1) ===============
                with (
                    tc.tile_pool(name="psS", bufs=2, space="PSUM") as psS,
                    tc.tile_pool(name="psO", bufs=2, space="PSUM") as psO,
                ):
                    box["psT"] = (psS, "ps")
                    # wo weight loads (needed from ATT_3 onwards)
                    for h in range(HL):
                        wos[h] = wopool.tile([128, 8 * OC], bf, name="wo_sb")
                        nc.sync.dma_start(wos[h][:], wo_blk[h])
                    for h in range(3):
                        filler = emit_qkv_head(h + 1)
                        att_head(h, filler, psS, psO, last=False)
                        for _ in filler:   # finish QKV h+1
                            pass

            # =============== ATT_3 ⊗ output projection ===============
            with (
                tc.tile_pool(name="ag", bufs=1) as agpool,
                tc.tile_pool(name="yp", bufs=4) as yppool,
                tc.tile_pool(name="yout", bufs=2) as ypool,
                tc.tile_pool(name="psS2", bufs=2, space="PSUM") as psS2,
                tc.tile_pool(name="psO2", bufs=2, space="PSUM") as psO2,
            ):
                ag_sb = [None] * 3
                ag_sb3 = [None] * 2
                yparts = [None] * 4

                def outproj_gen():
                    for hh in range(3):
                        ag_sb[hh] = agpool.tile([128, 8 * S], bf, name="ag_sb",
                                                bufs=3)
                        nc.sync.dma_start(
                            ag_sb[hh][:].rearrange("p (r s) -> p r s", r=8),
                            ag_out[hh][:].rearrange("(r p) s -> p r s", p=128))
                    for oc8 in range(4):
                        ps = psp_pool.tile([128, S], f32, name="psp")
                        n = 0
                        for hh in range(3):
                            for r in range(8):
                                for scc in range(2):
                                    nc.tensor.matmul(
                                        ps[:, scc * 512:(scc + 1) * 512],
                                        wos[hh][:, (r * 4 + oc8) * 128:
                                                (r * 4 + oc8 + 1) * 128],
                                        ag_sb[hh][:, r * S + scc * 512:
                                                  r * S + (scc + 1) * 512],
                                        start=(n < 2), stop=(n >= 46))
                                    n += 1
                                    yield
                        yp_t = yppool.tile([128, S], bf, name="ypart")
                        nc.scalar.copy(yp_t[:], ps[:])
                        yparts[oc8] = yp_t

                filler = outproj_gen()
                att_head(3, filler, psS2, psO2, last=True)
                for _ in filler:     # finish h0-2 out-proj parts (covers AG3)
                    pass

                # gather head-3 halves
                for sc in range(2):
                    ag_sb3[sc] = agpool.tile([128, 8 * 512], bf, name="ag_sb3",
                                             bufs=2)
                    nc.sync.dma_start(
                        ag_sb3[sc][:].rearrange("p (r s) -> p r s", r=8),
                        ag_out3[sc][:].rearrange("(r p) s -> p r s", p=128))

                # head-3 contributions + final combine + store
                for oc8 in range(4):
                    ps = psp_pool.tile([128, S], f32, name="psp")
                    n = 0
                    for scc in range(2):
                        for r in range(8):
                            nc.tensor.matmul(
                                ps[:, scc * 512:(scc + 1) * 512],
                                wos[3][:, (r * 4 + oc8) * 128:
                                        (r * 4 + oc8 + 1) * 128],
                                ag_sb3[scc][:, r * 512:(r + 1) * 512],
                                start=(n % 8 == 0), stop=(n % 8 == 7))
                            n += 1
                    ysb = ypool.tile([128, S], f32, name="ysb")
                    nc.vector.tensor_add(ysb[:], ps[:], yparts[oc8][:])
                    nc.sync.dma_start(y[oc8 * 128:(oc8 + 1) * 128, :], ysb[:])

    nc.compile()
    return nc


def _host_prep(xs, cache_k, cache_v, norm_w, wq, wk, wv, wo):
    """Build the 8 per-core input maps (all layout work done on host)."""
    xs = np.asarray(xs, F32)
    cache_k = np.asarray(cache_k, F32)
    cache_v = np.asarray(cache_v, F32)
    norm_w = np.asarray(norm_w, F32)
    wq, wk, wv, wo = (np.asarray(w, F32) for w in (wq, wk, wv, wo))

    # xs^T tiled: [128, dk*S]
    xs_t = np.ascontiguousarray(
        xs.T.reshape(NDK, 128, S).transpose(1, 0, 2).reshape(128, NDK * S)
    ).astype(BF16)

    # RoPE tables (positions C..C+S-1), transposed [freq, s]
    half = HD // 2
    inv_freq = 1.0 / (THETA ** (np.arange(0, half, dtype=np.float64) * 2.0 / HD))
    pos = np.arange(S, dtype=np.float64) + C
    ang = np.outer(pos, inv_freq)          # [S, 64]
    cos1 = np.cos(ang).T.astype(F32)       # [64, S]
    sin1 = np.sin(ang).T.astype(F32)
    cosT = np.vstack([cos1, cos1]).astype(BF16)          # [128, S]
    sinT = np.vstack([-sin1, sin1]).astype(BF16)         # rotate-half signs

    # causal triangle for the diagonal 128x128 blocks: T[p, c] = 1 iff c >= p
    cc = np.arange(128)[None, :]
    pp = np.arange(128)[:, None]
    triW = (cc >= pp).astype(F32).astype(BF16)

    ones_d = np.ones((128, 128), F32).astype(BF16)
    id_d = np.eye(128, dtype=F32).astype(BF16)

    sc_q = F32(1.0) / np.sqrt(F32(HD))

    in_maps = []
    for c in range(N_CORES):
        osl = slice(OC * c, OC * (c + 1))
        hsl = slice(HL * c, HL * (c + 1))
        # fold norm_w into wq/wk/wv; fold 1/sqrt(HD) into wq
        wq_c = (wq[osl] * norm_w[None, :]) * sc_q   # [512, 4096]
        wk_c = wk[osl] * norm_w[None, :]
        wv_c = wv[osl] * norm_w[None, :]

        def col_layout(w_c):
            # [HL, 128, NDK*128]: [h, p, dk*128 + j] = w_c[h*128+j, dk*128+p]
            m = w_c.reshape(HL, 128, NDK, 128)          # [h, j, dk, p]
            return np.ascontiguousarray(
                m.transpose(0, 3, 2, 1).reshape(HL, 128, NDK * 128)).astype(BF16)

        wq_col = col_layout(wq_c)
        wk_col = col_layout(wk_c)
        wv_col = col_layout(wv_c)

        # wo block for y^T: [h, p, (r*4 + oc8)*128 + j] =
        #   wo[OC*c + oc8*128 + j, (4r + h)*128 + p]
        wo_c = wo[osl]                                  # [512, 4096]
        m = wo_c.reshape(4, 128, 8, HL, 128)            # [oc8, j, r, h, p]
        wo_blk = np.ascontiguousarray(
            m.transpose(3, 4, 2, 0, 1).reshape(HL, 128, 8 * OC)).astype(BF16)

        # cache K^T per head: [h, p(hd), t]
        ck = np.ascontiguousarray(
            cache_k[:, hsl, :].transpose(1, 2, 0)).astype(BF16)   # [HL, 128, C]
        # cache V tiles: [h, p(t%128), ti*128 + hd]
        cv = np.ascontiguousarray(
            cache_v[:, hsl, :].reshape(NTC, 128, HL, HD)
            .transpose(2, 1, 0, 3).reshape(HL, 128, C)).astype(BF16)

        in_maps.append({
            "xs_t": xs_t, "wq_col": wq_col, "wk_col": wk_col, "wv_col": wv_col,
            "wo_blk": wo_blk, "ckt": ck, "cvr": cv,
            "cosT": cosT, "sinT": sinT, "triW": triW,
            "ones_d": ones_d, "id_d": id_d,
        })
    return in_maps


_NC_CACHE = {}


def kernel(xs, cache_k, cache_v, norm_w, wq, wk, wv, wo, _trace=False):
    if "nc" not in _NC_CACHE:
        _NC_CACHE["nc"] = _build_nc()
    nc = _NC_CACHE["nc"]
    in_maps = _host_prep(xs, cache_k, cache_v, norm_w, wq, wk, wv, wo)
    res = bass_utils.run_bass_kernel_spmd(
        nc, in_maps, core_ids=list(range(N_CORES)), trace=_trace)
    out = np.concatenate(
        [res.results[c]["y"].T for c in range(N_CORES)], axis=1)
    out = np.ascontiguousarray(out)
    if _trace:
        kernel.last_exec_time_ns = res.exec_time_ns
        kernel.last_results = res
    return out


# revision 9
# speedup vs baseline: 1.1521x; 1.0343x over previous
"""Trainium2 Bass kernel for a single-layer attention module (RMSNorm + QKV +
RoPE + causal attention over a KV cache + output projection), tensor-parallel
over 8 NeuronCores (4 heads each), per-head AllGather of attention outputs,
and per-core output-column blocks of the final projection.

Merged-pipeline version: QKV projections of head h+1 and output-projection
matmuls are interleaved into attention head h's PE stream (hiding exp latency
and the collectives), softmax denominators are accumulated on the vector
engine (pair chains) instead of ones-matmuls, RMSNorm statistics are
accumulated on the vector engine, and score/PV matmuls use exact causal
widths.

Self-contained: takes FULL inputs, returns the FULL [1024, 4096] f32 output.
"""

import sys

sys.path.insert(0, "/opt/trn_rl_repo")

from collections import deque
from itertools import islice

import numpy as np
import ml_dtypes

import concourse.bass as bass  # noqa: F401
import concourse.bacc as bacc
import concourse.tile as tile
from concourse import mybir
from concourse import bass_utils

BF16 = ml_dtypes.bfloat16
F32 = np.float32

N_CORES = 8
D, H, HD, S, C = 4096, 32, 128, 1024, 2048
T = C + S          # 3072 total keys
HL = H // N_CORES  # 4 heads per core
OC = HL * HD       # 512 local attention features per core
NDK = D // 128     # 32 contraction tiles over D
NTC = C // 128     # 16 cache t-tiles
NTN = S // 128     # 8 new-key t-tiles
EPS = 1e-6
THETA = 10000.0

bf = mybir.dt.bfloat16
f32 = mybir.dt.float32


def _build_nc():
    nc = bacc.Bacc("TRN2", target_bir_lowering=False, debug=False,
                   num_devices=N_CORES)

    # ---- DRAM I/O ----
    xs_t = nc.dram_tensor("xs_t", [128, NDK * S], bf, kind="ExternalInput")
    wq_col = nc.dram_tensor("wq_col", [HL, 128, NDK * 128], bf, kind="ExternalInput")
    wk_col = nc.dram_tensor("wk_col", [HL, 128, NDK * 128], bf, kind="ExternalInput")
    wv_col = nc.dram_tensor("wv_col", [HL, 128, NDK * 128], bf, kind="ExternalInput")
    wo_blk = nc.dram_tensor("wo_blk", [HL, 128, 8 * OC], bf, kind="ExternalInput")
    ckt = nc.dram_tensor("ckt", [HL, 128, C], bf, kind="ExternalInput")
    cvr = nc.dram_tensor("cvr", [HL, 128, C], bf, kind="ExternalInput")
    cosT = nc.dram_tensor("cosT", [128, S], bf, kind="ExternalInput")
    sinT = nc.dram_tensor("sinT", [128, S], bf, kind="ExternalInput")
    triW = nc.dram_tensor("triW", [128, 128], bf, kind="ExternalInput")
    ones_d = nc.dram_tensor("ones_d", [128, 128], bf, kind="ExternalInput")
    id_d = nc.dram_tensor("id_d", [128, 128], bf, kind="ExternalInput")
    # y stored transposed ([outcol, s]); host transposes back
    y = nc.dram_tensor("y", [OC, S], f32, kind="ExternalOutput")

    with tile.TileContext(nc) as tc:
        with (
            tc.tile_pool(name="const", bufs=1) as cpool,
            tc.tile_pool(name="qk", bufs=2) as qkpool,
            tc.tile_pool(name="att", bufs=2) as apool,
            tc.tile_pool(name="kv", bufs=2) as kvpool,
            tc.tile_pool(name="exp", bufs=6) as epool,
            tc.tile_pool(name="accp", bufs=6) as accpool,
            tc.tile_pool(name="rec", bufs=2) as recpool,
            tc.tile_pool(name="wo", bufs=4) as wopool,
            tc.tile_pool(name="dram", bufs=1, space="DRAM") as dpool,
            tc.tile_pool(name="psp", bufs=2, space="PSUM") as psp_pool,
        ):
            # ---- constants ----
            ones_t = cpool.tile([128, 128], bf, name="ones_t")
            nc.sync.dma_start(ones_t[:], ones_d[:])
            id_t = cpool.tile([128, 128], bf, name="id_t")
            nc.sync.dma_start(id_t[:], id_d[:])
            tri_t = cpool.tile([128, 128], bf, name="tri_t")
            nc.sync.dma_start(tri_t[:], triW[:])

            # per-head tile handles (ring depth 2: head h live + h+1 building)
            qr = [None] * HL
            kr = [None] * HL
            vsb = [None] * HL
            cks = [None] * HL
            cvs = [None] * HL
            wos = [None] * HL
            # collective dram buffers
            ag_in = [dpool.tile([128, S], bf, name=f"ag_in{h}") for h in range(3)]
            ag_out = [dpool.tile([N_CORES * 128, S], bf, name=f"ag_out{h}",
                                 addr_space="Shared") for h in range(3)]
            ag_in3 = [dpool.tile([128, 512], bf, name=f"ag_in3_{s}")
                      for s in range(2)]
            ag_out3 = [dpool.tile([N_CORES * 128, 512], bf, name=f"ag_out3_{s}",
                                  addr_space="Shared") for s in range(2)]

            box = {}       # rsq tile, transpose pool, transpose tile-name

            def groups_for(sc):
                # (kind, idx, off0): cache tiles full width, then new-key tiles
                # at exact causal width
                gs = [("c", ti, 0) for ti in range(NTC)]
                for tn in range(NTN):
                    if tn * 128 < (sc + 1) * 512:
                        gs.append(("n", tn, max(0, tn * 128 - sc * 512)))
                return gs

            def att_head(h, filler, psS, psO, last):
                """Emit attention for head h, draining filler between groups."""
                if h + 1 < HL:
                    cks[h + 1] = kvpool.tile([128, C], bf, name="ck_sb")
                    nc.sync.dma_start(cks[h + 1][:], ckt[h + 1])
                    cvs[h + 1] = kvpool.tile([128, C], bf, name="cv_sb")
                    nc.sync.dma_start(cvs[h + 1][:], cvr[h + 1])
                attnT = apool.tile([128, S], bf, name="attnT")

                def drain(k):
                    for _ in range(k):
                        if next(filler, None) is None:
                            break

                for sc in range(2):
                    drain_k = (5 if not last else (8 if sc == 0 else 1))
                    gs = groups_for(sc)
                    oc_t = psO.tile([128, 512], f32, name="oc")
                    partials = [None, None]
                    stash = [None, None]
                    nnew = [0]

                    def retire(g, e):
                        kind, idx, off0 = g
                        lv = (cvs[h][:, idx * 128:(idx + 1) * 128]
                              if kind == "c" else
                              vsb[h][:, idx * 128:(idx + 1) * 128])
                        nc.tensor.matmul(
                            oc_t[:, off0:512], lv, e[:, off0:512],
                            start=(g == gs[0]), stop=(g == gs[-1]))
                        # softmax denominator accumulation on DVE
                        if kind == "c":
                            j = idx % 2
                            if partials[j] is None:
                                if stash[j] is None:
                                    stash[j] = e
                                else:
                                    p = accpool.tile([128, 512], bf, name="accp")
                                    nc.vector.tensor_add(p[:], stash[j][:], e[:])
                                    partials[j] = p
                                    stash[j] = None
                            else:
                                nc.vector.tensor_add(
                                    partials[j][:], partials[j][:], e[:])
                        else:
                            j = nnew[0] % 2
                            nnew[0] += 1
                            nc.vector.tensor_add(
                                partials[j][:, off0:512],
                                partials[j][:, off0:512], e[:, off0:512])

                    pend = deque()
                    for g in gs:
                        kind, idx, off0 = g
                        lk = (cks[h][:, idx * 128:(idx + 1) * 128]
                              if kind == "c" else
                              kr[h][:, idx * 128:(idx + 1) * 128])
                        ps = psS.tile([128, 512], f32, name="ps")
                        nc.tensor.matmul(
                            ps[:, off0:512], lk,
                            qr[h][:, sc * 512 + off0:(sc + 1) * 512],
                            start=True, stop=True)
                        e = epool.tile([128, 512], bf, name="e")
                        nc.scalar.activation(
                            e[:, off0:512], ps[:, off0:512],
                            mybir.ActivationFunctionType.Exp)
                        if kind == "n" and idx * 128 >= sc * 512:
                            nc.vector.tensor_mul(
                                e[:, off0:off0 + 128], e[:, off0:off0 + 128],
                                tri_t[:])
                        drain(drain_k)
                        if len(pend) >= 2:
                            retire(*pend.popleft())
                        pend.append((g, e))
                    while pend:
                        retire(*pend.popleft())

                    # den = colsum(partial0) + colsum(partial1) via 2 ones-MMs
                    den_ps = psS.tile([128, 512], f32, name="ps")
                    nc.tensor.matmul(den_ps[:], ones_t[:], partials[0][:],
                                     start=True, stop=False)
                    nc.tensor.matmul(den_ps[:], ones_t[:], partials[1][:],
                                     start=False, stop=True)
                    rec = recpool.tile([128, 512], f32, name="rec")
                    nc.vector.reciprocal_approx_fast(rec[:], den_ps[:])
                    nc.vector.tensor_mul(
                        attnT[:, sc * 512:(sc + 1) * 512], oc_t[:], rec[:])
                    if not last:
                        nc.sync.dma_start(
                            ag_in[h][:, sc * 512:(sc + 1) * 512],
                            attnT[:, sc * 512:(sc + 1) * 512])
                    else:
                        nc.sync.dma_start(
                            ag_in3[sc][:], attnT[:, sc * 512:(sc + 1) * 512])
                        nc.gpsimd.collective_compute(
                            "AllGather", mybir.AluOpType.bypass,
                            replica_groups=[list(range(N_CORES))],
                            ins=[ag_in3[sc][:]], outs=[ag_out3[sc][:]])
                        if "ag3_gather" in box:
                            box["ag3_gather"](sc)
                if not last:
                    nc.gpsimd.collective_compute(
                        "AllGather", mybir.AluOpType.bypass,
                        replica_groups=[list(range(N_CORES))],
                        ins=[ag_in[h][:]], outs=[ag_out[h][:]])

            # =============== pre-region: xs + QKV head 0 (+ norm) ===============
            with (
                tc.tile_pool(name="xs", bufs=16) as xpool,
                tc.tile_pool(name="wcol", bufs=2) as wpool,
                tc.tile_pool(name="hh", bufs=2) as hpool,
                tc.tile_pool(name="rope", bufs=1) as rpool,
            ):
                # first weight column so PE work starts ASAP, then xs
                wcol0 = wpool.tile([128, NDK * 128], bf, name="wcol")
                nc.sync.dma_start(wcol0[:], wq_col[0])
                xs_ch = []
                for g in range(16):
                    xc = xpool.tile([128, 2 * S], bf, name="xs_ch")
                    nc.sync.dma_start(xc[:], xs_t[:, g * 2 * S:(g + 1) * 2 * S])
                    xs_ch.append(xc)

                def xs_v(dk):  # [128, S] view of raw xs^T d-tile dk
                    return xs_ch[dk // 2][:, (dk % 2) * S:(dk % 2 + 1) * S]

                cos_t = cpool.tile([128, S], bf, name="cos_t")
                nc.sync.dma_start(cos_t[:], cosT[:])
                sin_t = cpool.tile([128, S], bf, name="sin_t")
                nc.sync.dma_start(sin_t[:], sinT[:])
                # head-0 caches early (needed right at ATT_0)
                cks[0] = kvpool.tile([128, C], bf, name="ck_sb")
                nc.sync.dma_start(cks[0][:], ckt[0])
                cvs[0] = kvpool.tile([128, C], bf, name="cv_sb")
                nc.sync.dma_start(cvs[0][:], cvr[0])

                def rope(dst, src):
                    # dst = src * cos2 + rot(src) * sin2
                    rot = rpool.tile([128, S], bf, name="rot")
                    nc.sync.dma_start(rot[0:64, :], src[64:128, :])
                    nc.sync.dma_start(rot[64:128, :], src[0:64, :])
                    ta = rpool.tile([128, S], bf, name="ta")
                    nc.vector.tensor_mul(ta[:], src[:], cos_t[:])
                    tb = rpool.tile([128, S], bf, name="tb")
                    nc.vector.tensor_mul(tb[:], rot[:], sin_t[:])
                    nc.vector.tensor_add(dst[:], ta[:], tb[:])

                def emit_qkv_head(h):
                    """Generator: emits QKV for head h, yields after each PE op."""
                    qr[h] = qkpool.tile([128, S], bf, name="qr")
                    kr[h] = qkpool.tile([128, S], bf, name="kr")
                    vsb[h] = qkpool.tile([128, S], bf, name="vsb")
                    for which, wsrc in (("q", wq_col), ("k", wk_col), ("v", wv_col)):
                        if which == "q" and h == 0:
                            wc = wcol0
                        else:
                            wc = wpool.tile([128, NDK * 128], bf, name="wcol")
                            nc.sync.dma_start(wc[:], wsrc[h])
                        psp = psp_pool.tile([128, S], f32, name="psp")
                        for dk in range(NDK):
                            if h == 0 and which == "q":
                                box["norm_dk"](dk)
                            for scc in range(2):
                                nc.tensor.matmul(
                                    psp[:, scc * 512:(scc + 1) * 512],
                                    wc[:, dk * 128:(dk + 1) * 128],
                                    xs_v(dk)[:, scc * 512:(scc + 1) * 512],
                                    start=(dk == 0), stop=(dk == NDK - 1))
                                yield
                        if h == 0 and which == "q":
                            box["rsq_emit"]()
                        hh2 = hpool.tile([128, S], bf, name="hh2")
                        nc.vector.tensor_mul(hh2[:], psp[:], box["rsq"][:])
                        if which == "q":
                            rope(qr[h][:], hh2)
                        elif which == "k":
                            rope(kr[h][:], hh2)
                        else:
                            psT, ptr_name = box["psT"]
                            for tj in range(NTN):
                                ptr = psT.tile([128, 128], bf, name=ptr_name)
                                nc.tensor.transpose(
                                    ptr[:], hh2[:, tj * 128:(tj + 1) * 128],
                                    id_t[:])
                                nc.scalar.copy(
                                    vsb[h][:, tj * 128:(tj + 1) * 128], ptr[:])
                                yield

                # ---- RMSNorm stats: DVE squares + PE ones-MM accumulation ----
                gen0 = emit_qkv_head(0)
                with (
                    tc.tile_pool(name="nrm", bufs=1) as npool,
                    tc.tile_pool(name="sq", bufs=4) as sqpool,
                    tc.tile_pool(name="psn", bufs=1, space="PSUM") as psn,
                ):
                    ps_ss_box = {}

                    def norm_dk(dk):
                        sqt = sqpool.tile([128, S], bf, name="sqt")
                        nc.vector.tensor_mul(sqt[:], xs_v(dk), xs_v(dk))
                        if dk == 0:
                            ps_ss_box["t"] = psn.tile([128, S], f32, name="ps_ss")
                        ps_ss = ps_ss_box["t"]
                        for scc in range(2):
                            nc.tensor.matmul(
                                ps_ss[:, scc * 512:(scc + 1) * 512],
                                ones_t[:], sqt[:, scc * 512:(scc + 1) * 512],
                                start=(dk == 0), stop=(dk == NDK - 1))

                    def rsq_emit():
                        ps_ss = ps_ss_box["t"]
                        ssum = npool.tile([128, S], f32, name="ssum")
                        nc.scalar.activation(ssum[:], ps_ss[:],
                                             mybir.ActivationFunctionType.Copy,
                                             bias=EPS, scale=1.0 / D)
                        rcp = npool.tile([128, S], f32, name="rcp")
                        nc.vector.reciprocal_approx_fast(rcp[:], ssum[:])
                        rsq = cpool.tile([128, S], bf, name="rsq")
                        nc.scalar.sqrt(rsq[:], rcp[:])
                        box["rsq"] = rsq

                    box["norm_dk"] = norm_dk
                    box["rsq_emit"] = rsq_emit
                    # stage 1: q-head-0 projection + norm stats (65 resumes
                    # cover the 64 q matmuls plus the rsq/epilogue emission)
                    for _ in islice(gen0, 65):
                        pass

                # stage 2: k0/v0 with a transient transpose PSUM pool
                with tc.tile_pool(name="psT0", bufs=2, space="PSUM") as psT0:
                    box["psT"] = (psT0, "ptr")
                    for _ in gen0:
                        pass

                # =============== heads 0-2: attention ⊗ QKV(h+1) ===============
                with (
                    tc.tile_pool(name="psS", bufs=2, space="PSUM") as psS,
                    tc.tile_pool(name="psO", bufs=2, space="PSUM") as psO,
                ):
                    box["psT"] = (psS, "ps")
                    # wo weight loads (needed from ATT_3 onwards)
                    for h in range(HL):
                        wos[h] = wopool.tile([128, 8 * OC], bf, name="wo_sb")
                        nc.sync.dma_start(wos[h][:], wo_blk[h])
                    for h in range(3):
                        filler = emit_qkv_head(h + 1)
                        att_head(h, filler, psS, psO, last=False)
                        for _ in filler:   # finish QKV h+1
                            pass

            # =============== ATT_3 ⊗ output projection ===============
            with (
                tc.tile_pool(name="ag", bufs=1) as agpool,
                tc.tile_pool(name="yp", bufs=4) as yppool,
                tc.tile_pool(name="yout", bufs=2) as ypool,
                tc.tile_pool(name="psS2", bufs=2, space="PSUM") as psS2,
                tc.tile_pool(name="psO2", bufs=2, space="PSUM") as psO2,
            ):
                ag_sb = [None] * 3
                ag_sb3 = [None] * 2
                yparts = [None] * 4

                def outproj_gen():
                    for hh in range(3):
                        ag_sb[hh] = agpool.tile([128, 8 * S], bf, name="ag_sb",
                                                bufs=3)
                        nc.sync.dma_start(
                            ag_sb[hh][:].rearrange("p (r s) -> p r s", r=8),
                            ag_out[hh][:].rearrange("(r p) s -> p r s", p=128))
                    for oc8 in range(4):
                        ps = psp_pool.tile([128, S], f32, name="psp")
                        n = 0
                        for hh in range(3):
                            for r in range(8):
                                for scc in range(2):
                                    nc.tensor.matmul(
                                        ps[:, scc * 512:(scc + 1) * 512],
                                        wos[hh][:, (r * 4 + oc8) * 128:
                                                (r * 4 + oc8 + 1) * 128],
                                        ag_sb[hh][:, r * S + scc * 512:
                                                  r * S + (scc + 1) * 512],
                                        start=(n < 2), stop=(n >= 46))
                                    n += 1
                                    yield
                        yp_t = yppool.tile([128, S], bf, name="ypart")
                        nc.scalar.copy(yp_t[:], ps[:])
                        yparts[oc8] = yp_t

                def ag3_gather(sc):
                    ag_sb3[sc] = agpool.tile([128, 8 * 512], bf, name="ag_sb3",
                                             bufs=2)
                    nc.sync.dma_start(
                        ag_sb3[sc][:].rearrange("p (r s) -> p r s", r=8),
                        ag_out3[sc][:].rearrange("(r p) s -> p r s", p=128))

                box["ag3_gather"] = ag3_gather
                filler = outproj_gen()
                att_head(3, filler, psS2, psO2, last=True)
                for _ in filler:     # finish h0-2 out-proj parts (covers AG3)
                    pass

                # head-3 contributions + final combine + store
                for oc8 in range(4):
                    ps = psp_pool.tile([128, S], f32, name="psp")
                    n = 0
                    for scc in range(2):
                        for r in range(8):
                            nc.tensor.matmul(
                                ps[:, scc * 512:(scc + 1) * 512],
                                wos[3][:, (r * 4 + oc8) * 128:
                                        (r * 4 + oc8 + 1) * 128],
                                ag_sb3[scc][:, r * 512:(r + 1) * 512],
                                start=(n % 8 == 0), stop=(n % 8 == 7))
                            n += 1
                    ysb = ypool.tile([128, S], f32, name="ysb")
                    nc.vector.tensor_add(ysb[:], ps[:], yparts[oc8][:])
                    nc.sync.dma_start(y[oc8 * 128:(oc8 + 1) * 128, :], ysb[:])

    nc.compile()
    return nc


def _host_prep(xs, cache_k, cache_v, norm_w, wq, wk, wv, wo):
    """Build the 8 per-core input maps (all layout work done on host)."""
    xs = np.asarray(xs, F32)
    cache_k = np.asarray(cache_k, F32)
    cache_v = np.asarray(cache_v, F32)
    norm_w = np.asarray(norm_w, F32)
    wq, wk, wv, wo = (np.asarray(w, F32) for w in (wq, wk, wv, wo))

    # xs^T tiled: [128, dk*S]
    xs_t = np.ascontiguousarray(
        xs.T.reshape(NDK, 128, S).transpose(1, 0, 2).reshape(128, NDK * S)
    ).astype(BF16)

    # RoPE tables (positions C..C+S-1), transposed [freq, s]
    half = HD // 2
    inv_freq = 1.0 / (THETA ** (np.arange(0, half, dtype=np.float64) * 2.0 / HD))
    pos = np.arange(S, dtype=np.float64) + C
    ang = np.outer(pos, inv_freq)          # [S, 64]
    cos1 = np.cos(ang).T.astype(F32)       # [64, S]
    sin1 = np.sin(ang).T.astype(F32)
    cosT = np.vstack([cos1, cos1]).astype(BF16)          # [128, S]
    sinT = np.vstack([-sin1, sin1]).astype(BF16)         # rotate-half signs

    # causal triangle for the diagonal 128x128 blocks: T[p, c] = 1 iff c >= p
    cc = np.arange(128)[None, :]
    pp = np.arange(128)[:, None]
    triW = (cc >= pp).astype(F32).astype(BF16)

    ones_d = np.ones((128, 128), F32).astype(BF16)
    id_d = np.eye(128, dtype=F32).astype(BF16)

    sc_q = F32(1.0) / np.sqrt(F32(HD))

    in_maps = []
    for c in range(N_CORES):
        osl = slice(OC * c, OC * (c + 1))
        hsl = slice(HL * c, HL * (c + 1))
        # fold norm_w into wq/wk/wv; fold 1/sqrt(HD) into wq
        wq_c = (wq[osl] * norm_w[None, :]) * sc_q   # [512, 4096]
        wk_c = wk[osl] * norm_w[None, :]
        wv_c = wv[osl] * norm_w[None, :]

        def col_layout(w_c):
            # [HL, 128, NDK*128]: [h, p, dk*128 + j] = w_c[h*128+j, dk*128+p]
            m = w_c.reshape(HL, 128, NDK, 128)          # [h, j, dk, p]
            return np.ascontiguousarray(
                m.transpose(0, 3, 2, 1).reshape(HL, 128, NDK * 128)).astype(BF16)

        wq_col = col_layout(wq_c)
        wk_col = col_layout(wk_c)
        wv_col = col_layout(wv_c)

        # wo block for y^T: [h, p, (r*4 + oc8)*128 + j] =
        #   wo[OC*c + oc8*128 + j, (4r + h)*128 + p]
        wo_c = wo[osl]                                  # [512, 4096]
        m = wo_c.reshape(4, 128, 8, HL, 128)            # [oc8, j, r, h, p]
        wo_blk = np.ascontiguousarray(
            m.transpose(3, 4, 2, 0, 1).reshape(HL, 128, 8 * OC)).astype(BF16)

        # cache K^T per head: [h, p(hd), t]
        ck = np.ascontiguousarray(
            cache_k[:, hsl, :].transpose(1, 2, 0)).astype(BF16)   # [HL, 128, C]
        # cache V tiles: [h, p(t%128), ti*128 + hd]
        cv = np.ascontiguousarray(
            cache_v[:, hsl, :].reshape(NTC, 128, HL, HD)
            .transpose(2, 1, 0, 3).reshape(HL, 128, C)).astype(BF16)

        in_maps.append({
            "xs_t": xs_t, "wq_col": wq_col, "wk_col": wk_col, "wv_col": wv_col,
            "wo_blk": wo_blk, "ckt": ck, "cvr": cv,
            "cosT": cosT, "sinT": sinT, "triW": triW,
            "ones_d": ones_d, "id_d": id_d,
        })
    return in_maps


_NC_CACHE = {}


def kernel(xs, cache_k, cache_v, norm_w, wq, wk, wv, wo, _trace=False):
    if "nc" not in _NC_CACHE:
        _NC_CACHE["nc"] = _build_nc()
    nc = _NC_CACHE["nc"]
    in_maps = _host_prep(xs, cache_k, cache_v, norm_w, wq, wk, wv, wo)
    res = bass_utils.run_bass_kernel_spmd(
        nc, in_maps, core_ids=list(range(N_CORES)), trace=_trace)
    out = np.concatenate(
        [res.results[c]["y"].T for c in range(N_CORES)], axis=1)
    out = np.ascontiguousarray(out)
    if _trace:
        kernel.last_exec_time_ns = res.exec_time_ns
        kernel.last_results = res
    return out
